# revision 1
# baseline (speedup 1.0000x reference)
"""Trainium2 Bass kernel for a post-norm transformer encoder layer with RoPE.

B=2, S=2048, D=1024, H=16, Dh=64, F=4096, fp32 (f32r matmuls).

Sharding (8 cores, no collectives): core c handles batch b=c//4 and query block
qb=c%4 (512 queries). Each core recomputes K/V for its full batch (replicated
x4 within a batch group), computes Q/attention/out_proj/LN/FFN for its own 512
rows. Everything runs in transposed layout (features on partitions):
  - K^T, Q^T via stationary-W / moving-x^T matmuls; RoPE applied with
    host-built replicated cos / signed-sin tables + SBUF->SBUF DMA 32-row swaps.
  - V computed directly in (s, dh) layout (stationary x^T-slices, moving Wv^T),
    stored into V_aug tiles with a ones column per head (65-col stride) so the
    attention matmul also produces the softmax denominator.
  - scores computed transposed (keys on partitions), exp with no max-subtract
    (scores ~ N(0,1), safe in fp32), probs stay in SBUF as matmul moving side.
  - LayerNorm partition-dim stats via ones-column matmuls into PSUM.
DMA count is minimized (one packed const load, host-packed w1/w2 block loads)
because HWDGE descriptor prep serializes at ~0.6us per DMA.
"""
import sys, os
for _p in ('/opt/trn_rl_repo', '/root/.axon_site/_ro/trn_rl_repo'):
    if os.path.isdir(_p) and _p not in sys.path:
        sys.path.insert(0, _p)

import numpy as np
from contextlib import ExitStack

import concourse.bacc as bacc
import concourse.mybir as mybir
import concourse.tile as tile
from concourse.bass_utils import run_bass_kernel_spmd

F32 = mybir.dt.float32
F32R = mybir.dt.float32r
AF = mybir.ActivationFunctionType
ALU = mybir.AluOpType

B, S, D, H, Dh, F = 2, 2048, 1024, 16, 64, 4096
Q = 512                 # queries per core
NT_D = D // 128         # 8 d-tiles
NT_S = S // 128         # 16 s-tiles
NC_S = S // 512         # 4 s-chunks
NT_F = F // 128         # 32 f-tiles
LN_EPS = 1e-5
ROPE_BASE = 10000.0

# bpack column layout (each vector of length 1024 -> 8 cols, one per d-tile)
_BP = {"bq": 0, "bk": 8, "ob": 16, "b2": 24, "g1": 32, "be1": 40,
       "g2": 48, "be2": 56, "b1": 64, "ones": 96}
BP_COLS = 112

_CACHE = {}


def _build(repeat=1):
    nc = bacc.Bacc("TRN2", target_bir_lowering=False, debug=False, num_devices=8)

    def inp(name, shape, dt=F32R):
        return nc.dram_tensor(name, list(shape), dt, kind="ExternalInput")

    xT = inp("xT", (D, S))            # x[b].T
    xTq = inp("xTq", (D, Q))          # query-block slice of x[b].T (also residual)
    wqT = inp("wqT", (D, D))          # rope-permuted cols
    wkT = inp("wkT", (D, D))
    wvT = inp("wvT", (D, D))
    owT = inp("owT", (D, D))
    w1P = inp("w1P", (128, NT_D * F))   # w1T packed: [p, kt*F + c]
    w2P = inp("w2P", (128, NT_F * D))   # w2T packed: [p, ft*D + c]
    bvr = inp("bvr", (1, D))            # v bias row (K=1 matmul operand)
    bpack = inp("bpack", (128, BP_COLS))
    cosK = inp("cosK", (128, S), F32)    # [cos;cos;cos;cos] blocks of 32
    sinDK = inp("sinDK", (128, S), F32)  # [-sin;+sin;-sin;+sin]
    cosQ = inp("cosQ", (128, Q), F32)    # per-core column slice
    sinDQ = inp("sinDQ", (128, Q), F32)
    onesrow = inp("onesrow", (1, 128))   # bias-broadcast stationary
    yT = nc.dram_tensor("yT", [D, Q], F32, kind="ExternalOutput")

    with tile.TileContext(nc) as tc, ExitStack() as octx:
        pconst = octx.enter_context(tc.tile_pool(name="pconst", bufs=1))

        bp = pconst.tile([128, BP_COLS], F32R, tag="bp")
        nc.sync.dma_start(bp[:], bpack.ap())
        ones_r = pconst.tile([1, 128], F32R, tag="ones_r")
        nc.sync.dma_start(ones_r[:], onesrow.ap())
        bvr_t = pconst.tile([1, D], F32R, tag="bvr")
        nc.sync.dma_start(bvr_t[:], bvr.ap())

        def bcol(key, i):          # (128,1) f32 bias view
            c = _BP[key] + i
            return bp[:, c:c + 1].bitcast(F32)

        ones_c = bp[:, _BP["ones"]:_BP["ones"] + 1]   # f32r stationary
        ones16 = bp[:, _BP["ones"]:_BP["ones"] + 16]  # f32r ones cols

        for _rep in range(repeat):
            s_right = ExitStack()
            s_kvq = ExitStack()
            pKT = s_kvq.enter_context(tc.tile_pool(name="pKT", bufs=NT_D))
            KT = [pKT.tile([128, S], F32R, tag="kt", name=f"KT{i}")
                  for i in range(NT_D)]

            # ============ Phase A: K^T proj + rope (full-S chunks) ============
            with ExitStack() as ctx:
                wp = ctx.enter_context(tc.tile_pool(name="wk", bufs=1))
                xp = ctx.enter_context(tc.tile_pool(name="xA", bufs=1))
                tbl = ctx.enter_context(tc.tile_pool(name="tblK", bufs=1))
                scr = ctx.enter_context(tc.tile_pool(name="scrA", bufs=1))
                psA = ctx.enter_context(tc.tile_pool(name="psA", bufs=1, space="PSUM"))
                x_t = []
                for kt in range(NT_D):
                    t = xp.tile([128, S], F32R, tag=f"x{kt}", name=f"xA{kt}")
                    x_t.append(t)
                wk_h = {}
                for half in range(2):
                    wk_h[half] = []
                    for kt in range(NT_D):
                        t = wp.tile([128, 512], F32R, tag=f"w{kt}", bufs=1,
                                    name=f"wkA{half}_{kt}")
                        wk_h[half].append(t)
                # critical first: wk half0, x sc0
                for kt in range(NT_D):
                    nc.sync.dma_start(
                        wk_h[0][kt][:], wkT.ap()[kt * 128:(kt + 1) * 128, 0:512])
                for kt in range(NT_D):
                    nc.sync.dma_start(
                        x_t[kt][:, 0:512],
                        xT.ap()[kt * 128:(kt + 1) * 128, 0:512])
                cosK_t = tbl.tile([128, S], F32, tag="cosK")
                nc.sync.dma_start(cosK_t[:], cosK.ap())
                sinDK_t = tbl.tile([128, S], F32, tag="sinDK")
                nc.sync.dma_start(sinDK_t[:], sinDK.ap())
                for sc in range(1, NC_S):
                    for kt in range(NT_D):
                        nc.sync.dma_start(
                            x_t[kt][:, sc * 512:(sc + 1) * 512],
                            xT.ap()[kt * 128:(kt + 1) * 128,
                                    sc * 512:(sc + 1) * 512])
                for half in range(2):
                    wk_t = wk_h[half]
                    if half == 1:
                        for kt in range(NT_D):
                            nc.sync.dma_start(
                                wk_t[kt][:],
                                wkT.ap()[kt * 128:(kt + 1) * 128, 512:1024])
                    for dt2 in range(4):
                        dt = half * 4 + dt2
                        pk = psA.tile([128, S], F32, tag=f"pk{dt % 2}", bufs=1,
                                      name=f"pkA{dt}")
                        for kt in range(NT_D):
                            for sc in range(NC_S):
                                nc.tensor.matmul(
                                    pk[:, sc * 512:(sc + 1) * 512],
                                    lhsT=wk_t[kt][:, dt2 * 128:(dt2 + 1) * 128],
                                    rhs=x_t[kt][:, sc * 512:(sc + 1) * 512],
                                    start=(kt == 0), stop=(kt == NT_D - 1))
                        raw = scr.tile([128, S], F32, tag="rraw", bufs=2,
                                       name=f"rwA{dt}")
                        nc.scalar.activation(raw[:], pk[:], AF.Identity,
                                             bias=bcol("bk", dt))
                        sw = scr.tile([128, S], F32, tag="rsw", bufs=2,
                                      name=f"swA{dt}")
                        for a, bb in ((0, 32), (64, 96)):
                            nc.sync.dma_start(sw[a:a + 32, :], raw[bb:bb + 32, :])
                            nc.sync.dma_start(sw[bb:bb + 32, :], raw[a:a + 32, :])
                        nc.vector.tensor_mul(raw[:], raw[:], cosK_t[:])
                        nc.vector.tensor_mul(sw[:], sw[:], sinDK_t[:])
                        nc.vector.tensor_add(KT[dt][:], raw[:], sw[:])

            # ============ Phase B: V proj into V_aug ============
            pVA = s_kvq.enter_context(tc.tile_pool(name="pVA", bufs=NT_S))
            VA = [pVA.tile([128, 16 * 65], F32R, tag="va", name=f"VA{i}")
                  for i in range(NT_S)]
            with ExitStack() as ctx:
                xs = ctx.enter_context(tc.tile_pool(name="xsB", bufs=1))
                wp1 = ctx.enter_context(tc.tile_pool(name="wv1", bufs=1))
                psB = ctx.enter_context(tc.tile_pool(name="psB", bufs=1, space="PSUM"))
                wv_n = []
                for n in range(2):
                    row = []
                    for kt in range(NT_D):
                        t = wp1.tile([128, 512], F32R, tag=f"wv{n}_{kt}",
                                     name=f"wv{n}_{kt}")
                        nc.sync.dma_start(
                            t[:], wvT.ap()[kt * 128:(kt + 1) * 128,
                                           n * 512:(n + 1) * 512])
                        row.append(t)
                    wv_n.append(row)
                for sg in range(NC_S):
                    xch = []
                    for kt in range(NT_D):
                        t = xs.tile([128, 512], F32R, tag=f"xs{kt}", bufs=2,
                                    name=f"xsB{sg}_{kt}")
                        nc.sync.dma_start(
                            t[:], xT.ap()[kt * 128:(kt + 1) * 128,
                                          sg * 512:(sg + 1) * 512])
                        xch.append(t)
                    for sl in range(4):
                        st = sg * 4 + sl
                        va3 = VA[st].rearrange("p (h c) -> p h c", c=65)
                        nc.scalar.activation(
                            va3[:, :, 64:65],
                            ones16.rearrange("p (h c) -> p h c", c=1),
                            AF.Identity)
                        pv = psB.tile([128, 1024], F32, tag="pv", bufs=3,
                                      name=f"pv{st}")
                        for n in range(2):
                            nc.tensor.matmul(
                                pv[:, n * 512:(n + 1) * 512],
                                lhsT=ones_r[:],
                                rhs=bvr_t[:, n * 512:(n + 1) * 512],
                                start=True, stop=False)
                            for kt in range(NT_D):
                                nc.tensor.matmul(
                                    pv[:, n * 512:(n + 1) * 512],
                                    lhsT=xch[kt][:, sl * 128:(sl + 1) * 128],
                                    rhs=wv_n[n][kt][:],
                                    start=False, stop=(kt == NT_D - 1))
                            nc.scalar.activation(
                                va3[:, n * 8:(n + 1) * 8, 0:64],
                                pv[:, n * 512:(n + 1) * 512]
                                .rearrange("p (h c) -> p h c", c=64),
                                AF.Identity)

            # ============ Phase C: Q^T proj + rope ============
            pQT = s_right.enter_context(tc.tile_pool(name="pQT", bufs=NT_D,
                                                     side="right"))
            QT = [pQT.tile([128, Q], F32R, tag="qt", name=f"QT{i}")
                  for i in range(NT_D)]
            with ExitStack() as ctx:
                wp = ctx.enter_context(tc.tile_pool(name="wq", bufs=1))
                xqp = ctx.enter_context(tc.tile_pool(name="xqC", bufs=1))
                tbl = ctx.enter_context(tc.tile_pool(name="tblQ", bufs=1))
                scr = ctx.enter_context(tc.tile_pool(name="scrC", bufs=1))
                psC = ctx.enter_context(tc.tile_pool(name="psC", bufs=1, space="PSUM"))
                xq_t = []
                for kt in range(NT_D):
                    t = xqp.tile([128, Q], F32R, tag=f"xq{kt}", name=f"xqC{kt}")
                    nc.sync.dma_start(t[:], xTq.ap()[kt * 128:(kt + 1) * 128, :])
                    xq_t.append(t)
                cosQ_t = tbl.tile([128, Q], F32, tag="cosQ")
                nc.sync.dma_start(cosQ_t[:], cosQ.ap())
                sinDQ_t = tbl.tile([128, Q], F32, tag="sinDQ")
                nc.sync.dma_start(sinDQ_t[:], sinDQ.ap())
                for half in range(2):
                    wq_t = []
                    for kt in range(NT_D):
                        t = wp.tile([128, 512], F32R, tag=f"wq{kt}", bufs=1,
                                    name=f"wqC{half}_{kt}")
                        nc.sync.dma_start(
                            t[:], wqT.ap()[kt * 128:(kt + 1) * 128,
                                           half * 512:(half + 1) * 512])
                        wq_t.append(t)
                    for dt2 in range(4):
                        dt = half * 4 + dt2
                        pq = psC.tile([128, Q], F32, tag=f"pq{dt % 4}", bufs=2,
                                      name=f"pqC{dt}")
                        for kt in range(NT_D):
                            nc.tensor.matmul(
                                pq[:],
                                lhsT=wq_t[kt][:, dt2 * 128:(dt2 + 1) * 128],
                                rhs=xq_t[kt][:],
                                start=(kt == 0), stop=(kt == NT_D - 1))
                        raw = scr.tile([128, Q], F32, tag="rraw", bufs=2,
                                       name=f"rwC{dt}")
                        nc.scalar.activation(raw[:], pq[:], AF.Identity,
                                             bias=bcol("bq", dt))
                        sw = scr.tile([128, Q], F32, tag="rsw", bufs=2,
                                      name=f"swC{dt}")
                        for a, bb in ((0, 32), (64, 96)):
                            nc.sync.dma_start(sw[a:a + 32, :], raw[bb:bb + 32, :])
                            nc.sync.dma_start(sw[bb:bb + 32, :], raw[a:a + 32, :])
                        nc.vector.tensor_mul(raw[:], raw[:], cosQ_t[:])
                        nc.vector.tensor_mul(sw[:], sw[:], sinDQ_t[:])
                        nc.vector.tensor_add(QT[dt][:], raw[:], sw[:])

            # ============ Phase D: attention per head ============
            pATT = s_right.enter_context(tc.tile_pool(name="pATT", bufs=NT_D,
                                                      side="right"))
            ATT = [pATT.tile([128, Q], F32R, tag="att", name=f"ATT{i}")
                   for i in range(NT_D)]
            with ExitStack() as ctx:
                ptp = ctx.enter_context(tc.tile_pool(name="ptp", bufs=1))
                nrm = ctx.enter_context(tc.tile_pool(name="nrm", bufs=1))
                psS = ctx.enter_context(tc.tile_pool(name="psS", bufs=1, space="PSUM"))
                psAt = ctx.enter_context(tc.tile_pool(name="psAt", bufs=1,
                                                      space="PSUM"))
                for h in range(H):
                    dt, po = h // 2, (h % 2) * 64
                    pa = psAt.tile([65, 512], F32, tag="pa", bufs=2, name=f"pa{h}")
                    for kcp in range(NT_S // 2):
                        ps_t = psS.tile([128, 1024], F32, tag="ps", bufs=2,
                                        name=f"ps{h}_{kcp}")
                        for half in range(2):
                            kc = kcp * 2 + half
                            nc.tensor.matmul(
                                ps_t[:, half * 512:(half + 1) * 512],
                                lhsT=KT[dt][po:po + 64, kc * 128:(kc + 1) * 128],
                                rhs=QT[dt][po:po + 64, :],
                                start=True, stop=True)
                        pt_t = ptp.tile([128, 1024], F32R, tag="pt", bufs=3,
                                        name=f"pt{h}_{kcp}")
                        nc.scalar.activation(pt_t[:], ps_t[:], AF.Exp, scale=0.125)
                        for half in range(2):
                            kc = kcp * 2 + half
                            nc.tensor.matmul(
                                pa[:],
                                lhsT=VA[kc][:, h * 65:h * 65 + 65],
                                rhs=pt_t[:, half * 512:(half + 1) * 512],
                                start=(kc == 0), stop=(kc == NT_S - 1))
                    rec = nrm.tile([1, 512], F32, tag="rec", bufs=2, name=f"rec{h}")
                    nc.scalar.activation(rec[:], pa[64:65, :], AF.Identity)
                    rec2 = nrm.tile([1, 512], F32, tag="rec2", bufs=2,
                                    name=f"rec2_{h}")
                    nc.vector.reciprocal(rec2[:], rec[:])
                    recb = nrm.tile([128, 512], F32, tag="recb", bufs=2,
                                    name=f"recb{h}")
                    nc.gpsimd.partition_broadcast(recb[:], rec2[:], channels=128)
                    if po == 0:
                        # psum + sbuf inputs, all base 0: normalize on DVE directly
                        nc.vector.tensor_mul(ATT[dt][0:64, :], pa[0:64, :],
                                             recb[0:64, :])
                    else:
                        nc.scalar.activation(ATT[dt][po:po + 64, :], pa[0:64, :],
                                             AF.Identity)
                        nc.vector.tensor_mul(ATT[dt][po:po + 64, :],
                                             ATT[dt][po:po + 64, :],
                                             recb[po:po + 64, :])
            s_kvq.close()   # free KT / VA

            # ============ Phase E: out_proj + residual + LN1 ============
            pH1 = s_right.enter_context(tc.tile_pool(name="pH1", bufs=NT_D,
                                                     side="right"))
            H1 = [pH1.tile([128, Q], F32R, tag="h1", name=f"H1_{i}")
                  for i in range(NT_D)]
            with ExitStack() as ctx:
                wp = ctx.enter_context(tc.tile_pool(name="wo", bufs=1))
                hrp = ctx.enter_context(tc.tile_pool(name="pHR", bufs=NT_D))
                xrp = ctx.enter_context(tc.tile_pool(name="xrE", bufs=1))
                scr = ctx.enter_context(tc.tile_pool(name="scrE", bufs=1))
                stat = ctx.enter_context(tc.tile_pool(name="statE", bufs=1))
                psE = ctx.enter_context(tc.tile_pool(name="psE", bufs=1, space="PSUM"))
                psStat = ctx.enter_context(tc.tile_pool(name="psStatE", bufs=1,
                                                        space="PSUM"))
                ow_t = []
                for at_ in range(NT_D):
                    t = wp.tile([128, D], F32R, tag=f"wo{at_}", name=f"wo{at_}")
                    for s2 in range(2):
                        nc.sync.dma_start(
                            t[:, s2 * 512:(s2 + 1) * 512],
                            owT.ap()[at_ * 128:(at_ + 1) * 128,
                                     s2 * 512:(s2 + 1) * 512])
                    ow_t.append(t)
                pSum = psStat.tile([1, Q], F32, tag="psum_s")
                pSq = psStat.tile([1, Q], F32, tag="psum_q")
                HR = [hrp.tile([128, Q], F32R, tag="hr", name=f"HR{i}")
                      for i in range(NT_D)]
                for ot in range(NT_D):
                    po_t = psE.tile([128, Q], F32, tag="po", bufs=2, name=f"poE{ot}")
                    for at_ in range(NT_D):
                        nc.tensor.matmul(po_t[:],
                                         lhsT=ow_t[at_][:, ot * 128:(ot + 1) * 128],
                                         rhs=ATT[at_][:],
                                         start=(at_ == 0), stop=(at_ == NT_D - 1))
                    ho = scr.tile([128, Q], F32, tag="ho", bufs=2, name=f"hoE{ot}")
                    nc.scalar.activation(ho[:], po_t[:], AF.Identity,
                                         bias=bcol("ob", ot))
                    xr = xrp.tile([128, Q], F32R, tag="xr", bufs=2, name=f"xrE{ot}")
                    nc.sync.dma_start(xr[:], xTq.ap()[ot * 128:(ot + 1) * 128, :])
                    nc.vector.tensor_add(HR[ot][:], ho[:], xr[:].bitcast(F32))
                    sq = scr.tile([128, Q], F32R, tag="sq", bufs=2, name=f"sqE{ot}")
                    nc.scalar.activation(sq[:], HR[ot][:].bitcast(F32), AF.Square)
                    nc.tensor.matmul(pSum[:], lhsT=ones_c, rhs=HR[ot][:],
                                     start=(ot == 0), stop=(ot == NT_D - 1))
                    nc.tensor.matmul(pSq[:], lhsT=ones_c, rhs=sq[:],
                                     start=(ot == 0), stop=(ot == NT_D - 1))
                mu = stat.tile([1, Q], F32, tag="mu")
                nc.vector.tensor_scalar_mul(mu[:], pSum[:], 1.0 / D)
                var = stat.tile([1, Q], F32, tag="var")
                nc.vector.tensor_scalar_mul(var[:], pSq[:], 1.0 / D)
                mu2 = stat.tile([1, Q], F32, tag="mu2")
                nc.vector.tensor_mul(mu2[:], mu[:], mu[:])
                nc.vector.tensor_sub(var[:], var[:], mu2[:])
                nc.vector.tensor_scalar_add(var[:], var[:], LN_EPS)
                sd = stat.tile([1, Q], F32, tag="sd")
                nc.scalar.activation(sd[:], var[:], AF.Sqrt)
                rstd = stat.tile([1, Q], F32, tag="rstd")
                nc.vector.reciprocal(rstd[:], sd[:])
                muf = stat.tile([128, Q], F32, tag="muf")
                nc.gpsimd.partition_broadcast(muf[:], mu[:], channels=128)
                rstdf = stat.tile([128, Q], F32, tag="rstdf")
                nc.gpsimd.partition_broadcast(rstdf[:], rstd[:], channels=128)
                for ot in range(NT_D):
                    t1 = scr.tile([128, Q], F32, tag="t1", bufs=2, name=f"t1E{ot}")
                    nc.vector.tensor_sub(t1[:], HR[ot][:].bitcast(F32), muf[:])
                    nc.vector.tensor_mul(t1[:], t1[:], rstdf[:])
                    nc.vector.tensor_scalar(H1[ot][:], t1[:], bcol("g1", ot),
                                            bcol("be1", ot), ALU.mult, ALU.add)

            # ============ Phase F: FFN + residual + LN2 ============
            with ExitStack() as ctx:
                ffp = ctx.enter_context(tc.tile_pool(name="pFF", bufs=NT_F))
                scr = ctx.enter_context(tc.tile_pool(name="scrF", bufs=1))
                stat = ctx.enter_context(tc.tile_pool(name="statF", bufs=1))
                grp = ctx.enter_context(tc.tile_pool(name="grp", bufs=NT_D))
                psF = ctx.enter_context(tc.tile_pool(name="psF", bufs=1, space="PSUM"))
                psG = ctx.enter_context(tc.tile_pool(name="psG", bufs=1, space="PSUM"))
                psStat = ctx.enter_context(tc.tile_pool(name="psStatF", bufs=1,
                                                        space="PSUM"))
                FFT = [ffp.tile([128, Q], F32R, tag="ff", name=f"FFT{i}")
                       for i in range(NT_F)]
                w1v = w1P.ap().rearrange("p (kt c) -> p kt c", c=F)
                with tc.tile_pool(name="w1p", bufs=1) as w1p:
                    for fb in range(F // 512):
                        w1b = w1p.tile([128, NT_D * 512], F32R, tag="w1", bufs=2,
                                       name=f"w1b{fb}")
                        nc.sync.dma_start(
                            w1b[:].rearrange("p (kt c) -> p kt c", c=512),
                            w1v[:, :, fb * 512:(fb + 1) * 512])
                        for j in range(4):
                            ft = fb * 4 + j
                            pf = psF.tile([128, Q], F32, tag="pf", bufs=2,
                                          name=f"pf{ft}")
                            for kt in range(NT_D):
                                nc.tensor.matmul(
                                    pf[:],
                                    lhsT=w1b[:, kt * 512 + j * 128:
                                             kt * 512 + (j + 1) * 128],
                                    rhs=H1[kt][:],
                                    start=(kt == 0), stop=(kt == NT_D - 1))
                            nc.scalar.activation(FFT[ft][:], pf[:], AF.Relu,
                                                 bias=bcol("b1", ft))
                pSum2 = psStat.tile([1, Q], F32, tag="psum2_s")
                pSq2 = psStat.tile([1, Q], F32, tag="psum2_q")
                GR = [grp.tile([128, Q], F32R, tag="gr", name=f"GR{i}")
                      for i in range(NT_D)]
                w2v = w2P.ap().rearrange("p (ft c) -> p ft c", c=D)
                with tc.tile_pool(name="w2p", bufs=1) as w2p:
                    for ot in range(NT_D):
                        w2b = w2p.tile([128, NT_F * 128], F32R, tag="w2", bufs=2,
                                       name=f"w2b{ot}")
                        nc.sync.dma_start(
                            w2b[:].rearrange("p (ft c) -> p ft c", c=128),
                            w2v[:, :, ot * 128:(ot + 1) * 128])
                        pg = psG.tile([128, Q], F32, tag="pg", bufs=2, name=f"pg{ot}")
                        for ft in range(NT_F):
                            nc.tensor.matmul(
                                pg[:], lhsT=w2b[:, ft * 128:(ft + 1) * 128],
                                rhs=FFT[ft][:],
                                start=(ft == 0), stop=(ft == NT_F - 1))
                        go = scr.tile([128, Q], F32, tag="go", bufs=2,
                                      name=f"goF{ot}")
                        nc.scalar.activation(go[:], pg[:], AF.Identity,
                                             bias=bcol("b2", ot))
                        nc.vector.tensor_add(GR[ot][:], go[:], H1[ot][:].bitcast(F32))
                        sq2 = scr.tile([128, Q], F32R, tag="sq2", bufs=2,
                                       name=f"sq2F{ot}")
                        nc.scalar.activation(sq2[:], GR[ot][:].bitcast(F32),
                                             AF.Square)
                        nc.tensor.matmul(pSum2[:], lhsT=ones_c, rhs=GR[ot][:],
                                         start=(ot == 0), stop=(ot == NT_D - 1))
                        nc.tensor.matmul(pSq2[:], lhsT=ones_c, rhs=sq2[:],
                                         start=(ot == 0), stop=(ot == NT_D - 1))
                mu = stat.tile([1, Q], F32, tag="mu")
                nc.vector.tensor_scalar_mul(mu[:], pSum2[:], 1.0 / D)
                var = stat.tile([1, Q], F32, tag="var")
                nc.vector.tensor_scalar_mul(var[:], pSq2[:], 1.0 / D)
                mu2 = stat.tile([1, Q], F32, tag="mu2")
                nc.vector.tensor_mul(mu2[:], mu[:], mu[:])
                nc.vector.tensor_sub(var[:], var[:], mu2[:])
                nc.vector.tensor_scalar_add(var[:], var[:], LN_EPS)
                sd = stat.tile([1, Q], F32, tag="sd")
                nc.scalar.activation(sd[:], var[:], AF.Sqrt)
                rstd = stat.tile([1, Q], F32, tag="rstd")
                nc.vector.reciprocal(rstd[:], sd[:])
                muf = stat.tile([128, Q], F32, tag="muf")
                nc.gpsimd.partition_broadcast(muf[:], mu[:], channels=128)
                rstdf = stat.tile([128, Q], F32, tag="rstdf")
                nc.gpsimd.partition_broadcast(rstdf[:], rstd[:], channels=128)
                for ot in range(NT_D):
                    t1 = scr.tile([128, Q], F32, tag="t1f", bufs=2, name=f"t1F{ot}")
                    nc.vector.tensor_sub(t1[:], GR[ot][:].bitcast(F32), muf[:])
                    nc.vector.tensor_mul(t1[:], t1[:], rstdf[:])
                    yt = scr.tile([128, Q], F32, tag="yt", bufs=2, name=f"ytF{ot}")
                    nc.vector.tensor_scalar(yt[:], t1[:], bcol("g2", ot),
                                            bcol("be2", ot), ALU.mult, ALU.add)
                    nc.sync.dma_start(yT.ap()[ot * 128:(ot + 1) * 128, :], yt[:])
            s_right.close()

    nc.compile()
    return nc


def _rope_tables():
    inv_freq = (1.0 / (ROPE_BASE ** (np.arange(0, Dh, 2, dtype=np.float32) / Dh)))
    angles = np.arange(S, dtype=np.float32)[:, None] * inv_freq[None, :]
    cos = np.cos(angles).T.astype(np.float32)   # (32, S)
    sin = np.sin(angles).T.astype(np.float32)
    cosK = np.concatenate([cos, cos, cos, cos], axis=0)          # (128, S)
    sinDK = np.concatenate([-sin, sin, -sin, sin], axis=0)
    return np.ascontiguousarray(cosK), np.ascontiguousarray(sinDK)


def _in_maps(x, in_proj_w, in_proj_b, out_w, out_b, w1, b1, w2, b2,
             ln1_g, ln1_b, ln2_g, ln2_b):
    x = np.asarray(x, dtype=np.float32)
    f32 = lambda a: np.ascontiguousarray(np.asarray(a, dtype=np.float32))

    perm = np.concatenate(
        [h * Dh + np.concatenate([np.arange(0, Dh, 2), np.arange(1, Dh, 2)])
         for h in range(H)])
    wq = np.asarray(in_proj_w)[0:D][perm]
    wk = np.asarray(in_proj_w)[D:2 * D][perm]
    wv = np.asarray(in_proj_w)[2 * D:3 * D]
    bqv = np.asarray(in_proj_b)[0:D][perm]
    bkv = np.asarray(in_proj_b)[D:2 * D][perm]
    bvv = np.asarray(in_proj_b)[2 * D:3 * D]
    cosK, sinDK = _rope_tables()

    w1T = np.asarray(w1, dtype=np.float32).T          # (D, F)
    w2T = np.asarray(w2, dtype=np.float32).T          # (F, D)
    w1Pm = np.ascontiguousarray(
        w1T.reshape(NT_D, 128, F).transpose(1, 0, 2).reshape(128, NT_D * F))
    w2Pm = np.ascontiguousarray(
        w2T.reshape(NT_F, 128, D).transpose(1, 0, 2).reshape(128, NT_F * D))

    bpack = np.zeros((128, BP_COLS), np.float32)

    def put(key, vec):
        v = np.asarray(vec, dtype=np.float32).reshape(-1)
        n = v.size // 128
        bpack[:, _BP[key]:_BP[key] + n] = v.reshape(n, 128).T
    put("bq", bqv); put("bk", bkv); put("ob", out_b); put("b2", b2)
    put("g1", ln1_g); put("be1", ln1_b); put("g2", ln2_g); put("be2", ln2_b)
    put("b1", b1)
    bpack[:, _BP["ones"]:_BP["ones"] + 16] = 1.0

    shared = {
        "wqT": f32(wq.T), "wkT": f32(wk.T), "wvT": f32(wv.T),
        "owT": f32(np.asarray(out_w).T),
        "w1P": w1Pm, "w2P": w2Pm,
        "bvr": f32(bvv[None, :]),
        "bpack": bpack,
        "cosK": cosK, "sinDK": sinDK,
        "onesrow": np.ones((1, 128), np.float32),
    }
    xTs = [f32(x[b_].T) for b_ in range(B)]
    in_maps = []
    for c in range(8):
        b_, qb = c // 4, c % 4
        q0 = qb * Q
        m = dict(shared)
        m["xT"] = xTs[b_]
        m["xTq"] = f32(xTs[b_][:, q0:q0 + Q])
        m["cosQ"] = f32(cosK[:, q0:q0 + Q])
        m["sinDQ"] = f32(sinDK[:, q0:q0 + Q])
        in_maps.append(m)
    return in_maps


def kernel(x, in_proj_w, in_proj_b, out_w, out_b, w1, b1, w2, b2,
           ln1_g, ln1_b, ln2_g, ln2_b):
    if "nc" not in _CACHE:
        _CACHE["nc"] = _build()
    nc = _CACHE["nc"]
    in_maps = _in_maps(x, in_proj_w, in_proj_b, out_w, out_b, w1, b1, w2, b2,
                       ln1_g, ln1_b, ln2_g, ln2_b)
    res = run_bass_kernel_spmd(nc, in_maps, core_ids=list(range(8)))
    out = np.empty((B, S, D), dtype=np.float32)
    for c in range(8):
        b_, qb = c // 4, c % 4
        out[b_, qb * Q:(qb + 1) * Q, :] = res.results[c]["yT"].T
    return out


def _build_noop():
    nc = bacc.Bacc("TRN2", target_bir_lowering=False, debug=False, num_devices=8)
    names = [("xT", (D, S)), ("xTq", (D, Q)), ("wqT", (D, D)), ("wkT", (D, D)),
             ("wvT", (D, D)), ("owT", (D, D)), ("w1P", (128, NT_D * F)),
             ("w2P", (128, NT_F * D)), ("bvr", (1, D)), ("bpack", (128, BP_COLS)),
             ("cosK", (128, S)), ("sinDK", (128, S)), ("cosQ", (128, Q)),
             ("sinDQ", (128, Q)), ("onesrow", (1, 128))]
    dts = {}
    for nm, shape in names:
        dts[nm] = nc.dram_tensor(nm, list(shape), F32, kind="ExternalInput")
    yT = nc.dram_tensor("yT", [D, Q], F32, kind="ExternalOutput")
    with tile.TileContext(nc) as tc, ExitStack() as ctx:
        sb = ctx.enter_context(tc.tile_pool(name="sb", bufs=1))
        t = sb.tile([128, 1], F32, tag="t")
        nc.sync.dma_start(t[:], dts["bpack"].ap()[:, 0:1])
        nc.sync.dma_start(yT.ap()[0:128, 0:1], t[:])
    nc.compile()
    return nc


def baseline_time(inputs, iters=3):
    """Min wall-clock of a do-nothing kernel with the same input transfers."""
    import time
    if "nc0" not in _CACHE:
        _CACHE["nc0"] = _build_noop()
    nc0 = _CACHE["nc0"]
    in_maps = _in_maps(**inputs)
    run_bass_kernel_spmd(nc0, in_maps, core_ids=list(range(8)))  # warm compile
    ts = []
    for _ in range(iters):
        t0 = time.time()
        run_bass_kernel_spmd(nc0, in_maps, core_ids=list(range(8)))
        ts.append(time.time() - t0)
    return min(ts)



# revision 10
# speedup vs baseline: 1.7390x; 1.7390x over previous
"""Trainium2 Bass kernel for a post-norm transformer encoder layer with RoPE.

B=2, S=2048, D=1024, H=16, Dh=64, F=4096, fp32 in/out.

Sharding (8 cores, no collectives): core c handles batch b=c//4 and query block
qb=c%4 (512 queries). Each core recomputes K/V for its full batch, computes
Q/attention/out_proj/LN/FFN for its own 512 rows.

Mixed precision (rel tol 2e-2; this config measures ~1.0e-2 end-to-end):
  - QKV projections: fp8e4 x and weights, DoubleRow matmuls (2 k-tiles of 128
    packed per matmul -> 0.5 cycles/row).
  - RoPE + scores: bf16 (1 cycle/row), baseline swap-DMA rope structure.
  - probs: exp(0.125*s - 4) via ACT directly to fp8e4 (range [~2^-9, 240] covers
    max |score| 9.04 on these fixed inputs); denominator via fp8 ones column in
    the V_aug tiles, so normalization cancels the -4 shift exactly.
  - attn@V: fp8 DoubleRow over key-chunk pairs (V_aug stored in paired layout,
    65+65 cols per head: [v(kc even)|ones ; v(kc odd)|ones]).
  - out_proj + FFN: bf16 weights/activations (h, relu, residuals bf16).
  - LayerNorm stats via bf16 ones-column matmuls into PSUM; final LN2 in f32.
Engine placement: exp dominates ACT (~16.8M exps); V_aug writes go to GPSIMD,
relu/residual-adds/LN to DVE (bf16 2x modes where operands allow).
"""
import sys, os
for _p in ('/opt/trn_rl_repo', '/root/.axon_site/_ro/trn_rl_repo'):
    if os.path.isdir(_p) and _p not in sys.path:
        sys.path.insert(0, _p)

import numpy as np
from contextlib import ExitStack

import concourse.bacc as bacc
import concourse.mybir as mybir
import concourse.tile as tile
from concourse.bass_utils import run_bass_kernel_spmd

F32 = mybir.dt.float32
F32R = mybir.dt.float32r
BF16 = mybir.dt.bfloat16
F8 = mybir.dt.float8e4
AF = mybir.ActivationFunctionType
ALU = mybir.AluOpType
DR = mybir.MatmulPerfMode.DoubleRow

NP_F8 = mybir.dt.np(F8)
NP_BF = mybir.dt.np(BF16)

B, S, D, H, Dh, F = 2, 2048, 1024, 16, 64, 4096
Q = 512                 # queries per core
NT_D = D // 128         # 8 d-tiles
NT_S = S // 128         # 16 s-tiles
NC_S = S // 512         # 4 s-chunks
NT_F = F // 128         # 32 f-tiles
NPR = 4                 # contraction pairs (8 k-tiles -> 4 DoubleRow pairs)
LN_EPS = 1e-5
ROPE_BASE = 10000.0
EXP_SHIFT = -4.0        # exp(0.125*s + EXP_SHIFT); cancels in normalization

# bpack column layout (f32): each vector of length 1024 -> 8 cols (one per
# d-tile); b1 (4096) -> 32 cols; ones32 = 32 columns of 1.0
_BP = {"bq": 0, "bk": 8, "ob": 16, "b2": 24, "g1": 32, "be1": 40,
       "g2": 48, "be2": 56, "b1": 64, "ones": 96, "eb": 128}
BP_COLS = 136

_CACHE = {}


def _build(repeat=1):
    nc = bacc.Bacc("TRN2", target_bir_lowering=False, debug=False, num_devices=8)

    def inp(name, shape, dt):
        return nc.dram_tensor(name, list(shape), dt, kind="ExternalInput")

    xP8 = inp("xP8", (128, NPR * 2 * S), F8)      # x[b].T pair-packed
    xQ8 = inp("xQ8", (128, NPR * 2 * Q), F8)      # query-block slice
    wqP8 = inp("wqP8", (128, NPR * 2 * D), F8)    # rope-permuted cols
    wkP8 = inp("wkP8", (128, NPR * 2 * D), F8)
    wvP8 = inp("wvP8", (128, NPR * 2 * D), F8)    # moving layout for V
    owB = inp("owB", (128, NT_D * D), BF16)       # out_w.T packed (at-major)
    w1P = inp("w1P", (128, NT_D * F), BF16)       # w1T packed: [p, kt*F + c]
    w2P = inp("w2P", (128, NT_F * D), BF16)       # w2T packed: [p, ft*D + c]
    xrB = inp("xrB", (128, NT_D * Q), BF16)       # residual x.T query block
    bvr = inp("bvr", (1, D), F32R)                # v bias row (K=1 matmul)
    bpack = inp("bpack", (128, BP_COLS), F32)
    cbf = inp("cbf", (128, 2), BF16)              # bf16 ones col (stats)
    cosK = inp("cosK", (128, S), BF16)            # [cos;cos;cos;cos] 32-blocks
    sinDK = inp("sinDK", (128, S), BF16)          # [-sin;+sin;-sin;+sin]
    cosQ = inp("cosQ", (128, Q), BF16)            # per-core column slice
    sinDQ = inp("sinDQ", (128, Q), BF16)
    onesrow = inp("onesrow", (1, 128), F32R)      # bias-broadcast stationary
    yT = nc.dram_tensor("yT", [D, Q], F32, kind="ExternalOutput")

    with tile.TileContext(nc) as tc, ExitStack() as octx:
        pconst = octx.enter_context(tc.tile_pool(name="pconst", bufs=1))

        bp = pconst.tile([128, BP_COLS], F32, tag="bp")
        nc.sync.dma_start(bp[:], bpack.ap())
        cb = pconst.tile([128, 2], BF16, tag="cb")
        nc.sync.dma_start(cb[:], cbf.ap())
        ones_r = pconst.tile([1, 128], F32R, tag="ones_r")
        nc.sync.dma_start(ones_r[:], onesrow.ap())
        bvr_t = pconst.tile([1, D], F32R, tag="bvr")
        nc.sync.dma_start(bvr_t[:], bvr.ap())

        def bcol(key, i):          # (128,1) f32 bias view
            c = _BP[key] + i
            return bp[:, c:c + 1]

        ones32 = bp[:, _BP["ones"]:_BP["ones"] + 32]
        ones_bf = cb[:, 0:1]       # bf16 stationary for stats matmuls

        for _rep in range(repeat):
            s_right = ExitStack()
            s_kvq = ExitStack()
            s_proj = ExitStack()

            # persistent K/V pools (allocated below transient proj pools)
            pKT = s_kvq.enter_context(tc.tile_pool(name="pKT", bufs=NT_D))
            KT = [pKT.tile([128, S], BF16, tag="kt", name=f"KT{i}")
                  for i in range(NT_D)]
            pVA = s_kvq.enter_context(tc.tile_pool(name="pVA", bufs=NT_S // 2))
            VA = [pVA.tile([128, H * 160], F8, tag="va", name=f"VA{i}")
                  for i in range(NT_S // 2)]

            # --- input loads (issue early; wk/xp first for phase A) ---
            wpool = s_proj.enter_context(tc.tile_pool(name="wpool", bufs=1))
            xp_t = wpool.tile([128, NPR * 2 * S], F8, tag="xp")
            nc.sync.dma_start(xp_t[:], xP8.ap())
            wk_t = wpool.tile([128, NPR * 2 * D], F8, tag="wk")
            nc.sync.dma_start(wk_t[:], wkP8.ap())
            tblK = s_proj.enter_context(tc.tile_pool(name="tblK", bufs=1))
            cosK_t = tblK.tile([128, S], BF16, tag="cosK")
            nc.sync.dma_start(cosK_t[:], cosK.ap())
            sinDK_t = tblK.tile([128, S], BF16, tag="sinDK")
            nc.sync.dma_start(sinDK_t[:], sinDK.ap())
            wv_t = wpool.tile([128, NPR * 2 * D], F8, tag="wv")
            nc.sync.dma_start(wv_t[:], wvP8.ap())
            wq_t = wpool.tile([128, NPR * 2 * D], F8, tag="wq")
            nc.sync.dma_start(wq_t[:], wqP8.ap())
            xq_t = wpool.tile([128, NPR * 2 * Q], F8, tag="xq")
            nc.sync.dma_start(xq_t[:], xQ8.ap())
            cosQ_t = tblK.tile([128, Q], BF16, tag="cosQ")
            nc.sync.dma_start(cosQ_t[:], cosQ.ap())
            sinDQ_t = tblK.tile([128, Q], BF16, tag="sinDQ")
            nc.sync.dma_start(sinDQ_t[:], sinDQ.ap())

            def pr_view(t, pr, n):     # [128, 2, n-range] pair view
                return t[:, pr * 2 * n:(pr + 1) * 2 * n].rearrange(
                    "p (i c) -> p i c", i=2)

            # ============ Phase A: K^T proj + rope ============
            with ExitStack() as ctx:
                scr = ctx.enter_context(tc.tile_pool(name="scrA", bufs=1))
                psA = ctx.enter_context(tc.tile_pool(name="psA", bufs=1,
                                                     space="PSUM"))
                for dt in range(NT_D):
                    pk = psA.tile([128, S], F32, tag=f"pk{dt % 2}", bufs=1,
                                  name=f"pkA{dt}")
                    for sc in range(NC_S):
                        for pr in range(NPR):
                            nc.tensor.matmul(
                                pk[:, sc * 512:(sc + 1) * 512],
                                lhsT=pr_view(wk_t, pr, D)[:, :,
                                                          dt * 128:(dt + 1) * 128],
                                rhs=pr_view(xp_t, pr, S)[:, :,
                                                         sc * 512:(sc + 1) * 512],
                                start=(pr == 0), stop=(pr == NPR - 1),
                                perf_mode=DR)
                    raw = scr.tile([128, S], BF16, tag="rraw", bufs=2,
                                   name=f"rwA{dt}")
                    nc.scalar.activation(raw[:], pk[:], AF.Identity,
                                         bias=bcol("bk", dt))
                    sw = scr.tile([128, S], BF16, tag="rsw", bufs=2,
                                  name=f"swA{dt}")
                    for a, bb in ((0, 32), (64, 96)):
                        nc.sync.dma_start(sw[a:a + 32, :], raw[bb:bb + 32, :])
                        nc.sync.dma_start(sw[bb:bb + 32, :], raw[a:a + 32, :])
                    nc.vector.tensor_mul(raw[:], raw[:], cosK_t[:])
                    nc.vector.tensor_mul(sw[:], sw[:], sinDK_t[:])
                    nc.vector.tensor_add(KT[dt][:], raw[:], sw[:])

            # ============ Phase B: V proj into paired V_aug ============
            with ExitStack() as ctx:
                psB = ctx.enter_context(tc.tile_pool(name="psB", bufs=1,
                                                     space="PSUM"))
                for kcp in range(NT_S // 2):
                    va3 = VA[kcp].rearrange("p (hp c) -> p hp c", c=80)
                    nc.scalar.activation(
                        va3[:, :, 64:65],
                        ones32.rearrange("p (h c) -> p h c", c=1),
                        AF.Identity)
                for sg in range(NC_S):
                    for sl in range(4):
                        st = sg * 4 + sl
                        kcp, par = st // 2, st % 2
                        va4 = VA[kcp].rearrange("p (h pc) -> p h pc", pc=160)
                        pv = psB.tile([128, 1024], F32, tag="pv", bufs=3,
                                      name=f"pv{st}")
                        for n in range(2):
                            nc.tensor.matmul(
                                pv[:, n * 512:(n + 1) * 512],
                                lhsT=ones_r[:],
                                rhs=bvr_t[:, n * 512:(n + 1) * 512],
                                start=True, stop=False)
                            for pr in range(NPR):
                                nc.tensor.matmul(
                                    pv[:, n * 512:(n + 1) * 512],
                                    lhsT=pr_view(xp_t, pr, S)[
                                        :, :, sg * 512 + sl * 128:
                                        sg * 512 + (sl + 1) * 128],
                                    rhs=pr_view(wv_t, pr, D)[
                                        :, :, n * 512:(n + 1) * 512],
                                    start=False, stop=(pr == NPR - 1),
                                    perf_mode=DR)
                            nc.vector.tensor_copy(
                                va4[:, n * 8:(n + 1) * 8,
                                    par * 80:par * 80 + 64],
                                pv[:, n * 512:(n + 1) * 512]
                                .rearrange("p (h c) -> p h c", c=64))

            # ============ Phase C: Q^T proj + rope ============
            pQT = s_right.enter_context(tc.tile_pool(name="pQT", bufs=NT_D,
                                                     side="right"))
            QT = [pQT.tile([128, Q], BF16, tag="qt", name=f"QT{i}")
                  for i in range(NT_D)]
            with ExitStack() as ctx:
                scr = ctx.enter_context(tc.tile_pool(name="scrC", bufs=1))
                psC = ctx.enter_context(tc.tile_pool(name="psC", bufs=1,
                                                     space="PSUM"))
                for dt in range(NT_D):
                    pq = psC.tile([128, Q], F32, tag=f"pq{dt % 4}", bufs=1,
                                  name=f"pqC{dt}")
                    for pr in range(NPR):
                        nc.tensor.matmul(
                            pq[:],
                            lhsT=pr_view(wq_t, pr, D)[:, :,
                                                      dt * 128:(dt + 1) * 128],
                            rhs=pr_view(xq_t, pr, Q)[:],
                            start=(pr == 0), stop=(pr == NPR - 1),
                            perf_mode=DR)
                    raw = scr.tile([128, Q], BF16, tag="rraw", bufs=2,
                                   name=f"rwC{dt}")
                    nc.scalar.activation(raw[:], pq[:], AF.Identity,
                                         bias=bcol("bq", dt))
                    sw = scr.tile([128, Q], BF16, tag="rsw", bufs=2,
                                  name=f"swC{dt}")
                    for a, bb in ((0, 32), (64, 96)):
                        nc.sync.dma_start(sw[a:a + 32, :], raw[bb:bb + 32, :])
                        nc.sync.dma_start(sw[bb:bb + 32, :], raw[a:a + 32, :])
                    nc.vector.tensor_mul(raw[:], raw[:], cosQ_t[:])
                    nc.vector.tensor_mul(sw[:], sw[:], sinDQ_t[:])
                    nc.vector.tensor_add(QT[dt][:], raw[:], sw[:])
            s_proj.close()   # free xp/w/xq/tables

            # prefetch E/F-phase tensors while attention runs
            pEF = s_right.enter_context(tc.tile_pool(name="pEF", bufs=1,
                                                     side="right"))
            ow_t = pEF.tile([128, NT_D * D], BF16, tag="ow")
            nc.sync.dma_start(ow_t[:], owB.ap())
            xr_t = pEF.tile([128, NT_D * Q], BF16, tag="xr")
            nc.sync.dma_start(xr_t[:], xrB.ap())

            # ============ Phase D: attention per head ============
            pATT = s_right.enter_context(tc.tile_pool(name="pATT", bufs=NT_D,
                                                      side="right"))
            ATT = [pATT.tile([128, Q], BF16, tag="att", name=f"ATT{i}")
                   for i in range(NT_D)]
            with ExitStack() as ctx:
                ptp = ctx.enter_context(tc.tile_pool(name="ptp", bufs=1))
                nrm = ctx.enter_context(tc.tile_pool(name="nrm", bufs=1))
                psS = ctx.enter_context(tc.tile_pool(name="psS", bufs=1,
                                                     space="PSUM"))
                psAt = ctx.enter_context(tc.tile_pool(name="psAt", bufs=1,
                                                      space="PSUM"))
                for h in range(H):
                    dt, po = h // 2, (h % 2) * 64
                    pa = psAt.tile([65, 512], F32, tag="pa", bufs=2,
                                   name=f"pa{h}")
                    vh0 = h * 160
                    for kcp in range(NT_S // 2):
                        ps_t = psS.tile([128, 1024], F32, tag="ps", bufs=2,
                                        name=f"ps{h}_{kcp}")
                        for half in range(2):
                            kc = kcp * 2 + half
                            nc.tensor.matmul(
                                ps_t[:, half * 512:(half + 1) * 512],
                                lhsT=KT[dt][po:po + 64,
                                            kc * 128:(kc + 1) * 128],
                                rhs=QT[dt][po:po + 64, :],
                                start=True, stop=True)
                        pt_t = ptp.tile([128, 1024], F8, tag="pt", bufs=3,
                                        name=f"pt{h}_{kcp}")
                        nc.scalar.activation(pt_t[:], ps_t[:], AF.Exp,
                                             bias=bcol("eb", 0), scale=0.125)
                        nc.tensor.matmul(
                            pa[:],
                            lhsT=VA[kcp][:, vh0:vh0 + 160]
                            .rearrange("p (i c) -> p i c", i=2)[:, :, 0:65],
                            rhs=pt_t[:].rearrange("p (i s) -> p i s", i=2),
                            start=(kcp == 0), stop=(kcp == NT_S // 2 - 1),
                            perf_mode=DR)
                    rec2 = nrm.tile([1, 512], F32, tag="rec2", bufs=2,
                                    name=f"rec2_{h}")
                    nc.vector.reciprocal(rec2[:], pa[64:65, :])
                    recb = nrm.tile([128, 512], F32, tag="recb", bufs=2,
                                    name=f"recb{h}")
                    nc.gpsimd.partition_broadcast(recb[:], rec2[:],
                                                  channels=128)
                    if po == 0:
                        nc.vector.tensor_mul(ATT[dt][0:64, :], pa[0:64, :],
                                             recb[0:64, :])
                    else:
                        nc.scalar.activation(ATT[dt][po:po + 64, :],
                                             pa[0:64, :], AF.Identity)
                        nc.vector.tensor_mul(ATT[dt][po:po + 64, :],
                                             ATT[dt][po:po + 64, :],
                                             recb[po:po + 64, :])
            s_kvq.close()   # free KT / VA

            # ============ Phase E: out_proj + residual + LN1 ============
            pH1 = s_right.enter_context(tc.tile_pool(name="pH1", bufs=NT_D,
                                                     side="right"))
            H1 = [pH1.tile([128, Q], BF16, tag="h1", name=f"H1_{i}")
                  for i in range(NT_D)]
            with ExitStack() as ctx:
                hrp = ctx.enter_context(tc.tile_pool(name="pHR", bufs=NT_D))
                scr = ctx.enter_context(tc.tile_pool(name="scrE", bufs=1))
                stat = ctx.enter_context(tc.tile_pool(name="statE", bufs=1))
                psE = ctx.enter_context(tc.tile_pool(name="psE", bufs=1,
                                                     space="PSUM"))
                psStat = ctx.enter_context(tc.tile_pool(name="psStatE", bufs=1,
                                                        space="PSUM"))
                pSum = psStat.tile([1, Q], F32, tag="psum_s")
                pSq = psStat.tile([1, Q], F32, tag="psum_q")
                HR = [hrp.tile([128, Q], BF16, tag="hr", name=f"HR{i}")
                      for i in range(NT_D)]
                for ot in range(NT_D):
                    po_t = psE.tile([128, Q], F32, tag="po", bufs=2,
                                    name=f"poE{ot}")
                    for at_ in range(NT_D):
                        nc.tensor.matmul(
                            po_t[:],
                            lhsT=ow_t[:, at_ * D + ot * 128:
                                      at_ * D + (ot + 1) * 128],
                            rhs=ATT[at_][:],
                            start=(at_ == 0), stop=(at_ == NT_D - 1))
                    nc.vector.scalar_tensor_tensor(
                        HR[ot][:], po_t[:], bcol("ob", ot),
                        xr_t[:, ot * Q:(ot + 1) * Q], ALU.add, ALU.add)
                    sq = scr.tile([128, Q], BF16, tag="sq", bufs=2,
                                  name=f"sqE{ot}")
                    nc.vector.tensor_mul(sq[:], HR[ot][:], HR[ot][:])
                    nc.tensor.matmul(pSum[:], lhsT=ones_bf, rhs=HR[ot][:],
                                     start=(ot == 0), stop=(ot == NT_D - 1))
                    nc.tensor.matmul(pSq[:], lhsT=ones_bf, rhs=sq[:],
                                     start=(ot == 0), stop=(ot == NT_D - 1))
                mu = stat.tile([1, Q], F32, tag="mu")
                nc.vector.tensor_scalar_mul(mu[:], pSum[:], 1.0 / D)
                var = stat.tile([1, Q], F32, tag="var")
                nc.vector.tensor_scalar_mul(var[:], pSq[:], 1.0 / D)
                mu2 = stat.tile([1, Q], F32, tag="mu2")
                nc.vector.tensor_mul(mu2[:], mu[:], mu[:])
                nc.vector.tensor_sub(var[:], var[:], mu2[:])
                nc.vector.tensor_scalar_add(var[:], var[:], LN_EPS)
                sd = stat.tile([1, Q], F32, tag="sd")
                nc.scalar.activation(sd[:], var[:], AF.Sqrt)
                rstd = stat.tile([1, Q], F32, tag="rstd")
                nc.vector.reciprocal(rstd[:], sd[:])
                muf = stat.tile([128, Q], F32, tag="muf")
                nc.gpsimd.partition_broadcast(muf[:], mu[:], channels=128)
                rstdf = stat.tile([128, Q], F32, tag="rstdf")
                nc.gpsimd.partition_broadcast(rstdf[:], rstd[:], channels=128)
                for ot in range(NT_D):
                    t1 = scr.tile([128, Q], BF16, tag="t1", bufs=2,
                                  name=f"t1E{ot}")
                    nc.vector.tensor_sub(t1[:], HR[ot][:], muf[:])
                    nc.vector.tensor_mul(t1[:], t1[:], rstdf[:])
                    nc.vector.tensor_scalar(H1[ot][:], t1[:], bcol("g1", ot),
                                            bcol("be1", ot), ALU.mult, ALU.add)

            # ============ Phase F: FFN + residual + LN2 ============
            with ExitStack() as ctx:
                ffp = ctx.enter_context(tc.tile_pool(name="pFF", bufs=NT_F))
                scr = ctx.enter_context(tc.tile_pool(name="scrF", bufs=1))
                stat = ctx.enter_context(tc.tile_pool(name="statF", bufs=1))
                grp = ctx.enter_context(tc.tile_pool(name="grp", bufs=NT_D))
                psF = ctx.enter_context(tc.tile_pool(name="psF", bufs=1,
                                                     space="PSUM"))
                psG = ctx.enter_context(tc.tile_pool(name="psG", bufs=1,
                                                     space="PSUM"))
                psStat = ctx.enter_context(tc.tile_pool(name="psStatF", bufs=1,
                                                        space="PSUM"))
                FFT = [ffp.tile([128, Q], BF16, tag="ff", name=f"FFT{i}")
                       for i in range(NT_F)]
                w1v = w1P.ap().rearrange("p (kt c) -> p kt c", c=F)
                with tc.tile_pool(name="w1p", bufs=1) as w1p:
                    for fb in range(F // 512):
                        w1b = w1p.tile([128, NT_D * 512], BF16, tag="w1",
                                       bufs=2, name=f"w1b{fb}")
                        nc.sync.dma_start(
                            w1b[:].rearrange("p (kt c) -> p kt c", c=512),
                            w1v[:, :, fb * 512:(fb + 1) * 512])
                        for j in range(4):
                            ft = fb * 4 + j
                            pf = psF.tile([128, Q], F32, tag="pf", bufs=2,
                                          name=f"pf{ft}")
                            for kt in range(NT_D):
                                nc.tensor.matmul(
                                    pf[:],
                                    lhsT=w1b[:, kt * 512 + j * 128:
                                             kt * 512 + (j + 1) * 128],
                                    rhs=H1[kt][:],
                                    start=(kt == 0), stop=(kt == NT_D - 1))
                            nc.vector.tensor_scalar(FFT[ft][:], pf[:],
                                                    bcol("b1", ft), 0.0,
                                                    ALU.add, ALU.max)
                pSum2 = psStat.tile([1, Q], F32, tag="psum2_s")
                pSq2 = psStat.tile([1, Q], F32, tag="psum2_q")
                GR = [grp.tile([128, Q], BF16, tag="gr", name=f"GR{i}")
                      for i in range(NT_D)]
                w2v = w2P.ap().rearrange("p (ft c) -> p ft c", c=D)
                with tc.tile_pool(name="w2p", bufs=1) as w2p:
                    for ot in range(NT_D):
                        w2b = w2p.tile([128, NT_F * 128], BF16, tag="w2",
                                       bufs=2, name=f"w2b{ot}")
                        nc.sync.dma_start(
                            w2b[:].rearrange("p (ft c) -> p ft c", c=128),
                            w2v[:, :, ot * 128:(ot + 1) * 128])
                        pg = psG.tile([128, Q], F32, tag="pg", bufs=2,
                                      name=f"pg{ot}")
                        for ft in range(NT_F):
                            nc.tensor.matmul(
                                pg[:], lhsT=w2b[:, ft * 128:(ft + 1) * 128],
                                rhs=FFT[ft][:],
                                start=(ft == 0), stop=(ft == NT_F - 1))
                        nc.vector.scalar_tensor_tensor(
                            GR[ot][:], pg[:], bcol("b2", ot), H1[ot][:],
                            ALU.add, ALU.add)
                        sq2 = scr.tile([128, Q], BF16, tag="sq2", bufs=2,
                                       name=f"sq2F{ot}")
                        nc.vector.tensor_mul(sq2[:], GR[ot][:], GR[ot][:])
                        nc.tensor.matmul(pSum2[:], lhsT=ones_bf, rhs=GR[ot][:],
                                         start=(ot == 0), stop=(ot == NT_D - 1))
                        nc.tensor.matmul(pSq2[:], lhsT=ones_bf, rhs=sq2[:],
                                         start=(ot == 0), stop=(ot == NT_D - 1))
                mu = stat.tile([1, Q], F32, tag="mu")
                nc.vector.tensor_scalar_mul(mu[:], pSum2[:], 1.0 / D)
                var = stat.tile([1, Q], F32, tag="var")
                nc.vector.tensor_scalar_mul(var[:], pSq2[:], 1.0 / D)
                mu2 = stat.tile([1, Q], F32, tag="mu2")
                nc.vector.tensor_mul(mu2[:], mu[:], mu[:])
                nc.vector.tensor_sub(var[:], var[:], mu2[:])
                nc.vector.tensor_scalar_add(var[:], var[:], LN_EPS)
                sd = stat.tile([1, Q], F32, tag="sd")
                nc.scalar.activation(sd[:], var[:], AF.Sqrt)
                rstd = stat.tile([1, Q], F32, tag="rstd")
                nc.vector.reciprocal(rstd[:], sd[:])
                muf = stat.tile([128, Q], F32, tag="muf")
                nc.gpsimd.partition_broadcast(muf[:], mu[:], channels=128)
                rstdf = stat.tile([128, Q], F32, tag="rstdf")
                nc.gpsimd.partition_broadcast(rstdf[:], rstd[:], channels=128)
                for ot in range(NT_D):
                    t1 = scr.tile([128, Q], F32, tag="t1f", bufs=2,
                                  name=f"t1F{ot}")
                    nc.vector.tensor_sub(t1[:], GR[ot][:], muf[:])
                    nc.vector.tensor_mul(t1[:], t1[:], rstdf[:])
                    yt = scr.tile([128, Q], F32, tag="yt", bufs=2,
                                  name=f"ytF{ot}")
                    nc.vector.tensor_scalar(yt[:], t1[:], bcol("g2", ot),
                                            bcol("be2", ot), ALU.mult, ALU.add)
                    nc.sync.dma_start(yT.ap()[ot * 128:(ot + 1) * 128, :],
                                      yt[:])
            s_right.close()

    nc.compile()
    return nc


def _rope_tables():
    inv_freq = (1.0 / (ROPE_BASE ** (np.arange(0, Dh, 2, dtype=np.float32) / Dh)))
    angles = np.arange(S, dtype=np.float32)[:, None] * inv_freq[None, :]
    cos = np.cos(angles).T.astype(np.float32)   # (32, S)
    sin = np.sin(angles).T.astype(np.float32)
    cosK = np.concatenate([cos, cos, cos, cos], axis=0)          # (128, S)
    sinDK = np.concatenate([-sin, sin, -sin, sin], axis=0)
    return (np.ascontiguousarray(cosK.astype(NP_BF)),
            np.ascontiguousarray(sinDK.astype(NP_BF)))


def _pack_pairs(mT, dtype=None):
    """(1024, N) f32 -> [128, 4*2*N]: col = pr*2N + i*N + c, row-tiles paired."""
    n = mT.shape[1]
    m = mT.astype(NP_F8) if dtype is None else mT.astype(dtype)
    r = m.reshape(8, 128, n)
    return np.ascontiguousarray(
        r.transpose(1, 0, 2).reshape(128, 8 * n))


def _in_maps(x, in_proj_w, in_proj_b, out_w, out_b, w1, b1, w2, b2,
             ln1_g, ln1_b, ln2_g, ln2_b):
    x = np.asarray(x, dtype=np.float32)

    perm = np.concatenate(
        [h * Dh + np.concatenate([np.arange(0, Dh, 2), np.arange(1, Dh, 2)])
         for h in range(H)])
    wq = np.asarray(in_proj_w)[0:D][perm]
    wk = np.asarray(in_proj_w)[D:2 * D][perm]
    wv = np.asarray(in_proj_w)[2 * D:3 * D]
    bqv = np.asarray(in_proj_b)[0:D][perm]
    bkv = np.asarray(in_proj_b)[D:2 * D][perm]
    bvv = np.asarray(in_proj_b)[2 * D:3 * D]
    cosK, sinDK = _rope_tables()

    wqP8 = _pack_pairs(np.asarray(wq, np.float32).T)    # (D, D) -> fp8 pack
    wkP8 = _pack_pairs(np.asarray(wk, np.float32).T)
    wvP8 = _pack_pairs(np.asarray(wv, np.float32).T)

    owT = np.asarray(out_w, np.float32).T               # (D, D)
    owB = np.ascontiguousarray(
        owT.reshape(NT_D, 128, D).transpose(1, 0, 2).reshape(128, NT_D * D)
        .astype(NP_BF))
    w1T = np.asarray(w1, dtype=np.float32).T            # (D, F)
    w2T = np.asarray(w2, dtype=np.float32).T            # (F, D)
    w1Pm = np.ascontiguousarray(
        w1T.reshape(NT_D, 128, F).transpose(1, 0, 2).reshape(128, NT_D * F)
        .astype(NP_BF))
    w2Pm = np.ascontiguousarray(
        w2T.reshape(NT_F, 128, D).transpose(1, 0, 2).reshape(128, NT_F * D)
        .astype(NP_BF))

    bpack = np.zeros((128, BP_COLS), np.float32)

    def put(key, vec):
        v = np.asarray(vec, dtype=np.float32).reshape(-1)
        n = v.size // 128
        bpack[:, _BP[key]:_BP[key] + n] = v.reshape(n, 128).T
    put("bq", bqv); put("bk", bkv); put("ob", out_b); put("b2", b2)
    put("g1", ln1_g); put("be1", ln1_b); put("g2", ln2_g); put("be2", ln2_b)
    put("b1", b1)
    bpack[:, _BP["ones"]:_BP["ones"] + 32] = 1.0
    bpack[:, _BP["eb"]] = EXP_SHIFT

    cbf = np.zeros((128, 2), NP_BF)
    cbf[:, 0] = 1.0

    shared = {
        "wqP8": wqP8, "wkP8": wkP8, "wvP8": wvP8,
        "owB": owB, "w1P": w1Pm, "w2P": w2Pm,
        "bvr": np.ascontiguousarray(
            np.asarray(bvv, np.float32)[None, :]),
        "bpack": bpack, "cbf": cbf,
        "cosK": cosK, "sinDK": sinDK,
        "onesrow": np.ones((1, 128), np.float32),
    }
    xT8 = [_pack_pairs(x[b_].T) for b_ in range(B)]         # [128, 8*S] fp8
    xTs = [np.asarray(x[b_].T, np.float32) for b_ in range(B)]
    in_maps = []
    for c in range(8):
        b_, qb = c // 4, c % 4
        q0 = qb * Q
        m = dict(shared)
        m["xP8"] = xT8[b_]
        # query-block slice of the pair-packed x (cols within each k-tile)
        xq = xT8[b_].reshape(128, 8, S)[:, :, q0:q0 + Q]
        m["xQ8"] = np.ascontiguousarray(xq.reshape(128, 8 * Q))
        m["xrB"] = np.ascontiguousarray(
            xTs[b_][:, q0:q0 + Q].reshape(NT_D, 128, Q)
            .transpose(1, 0, 2).reshape(128, NT_D * Q).astype(NP_BF))
        m["cosQ"] = np.ascontiguousarray(cosK[:, q0:q0 + Q])
        m["sinDQ"] = np.ascontiguousarray(sinDK[:, q0:q0 + Q])
        in_maps.append(m)
    return in_maps


def kernel(x, in_proj_w, in_proj_b, out_w, out_b, w1, b1, w2, b2,
           ln1_g, ln1_b, ln2_g, ln2_b):
    if "nc" not in _CACHE:
        _CACHE["nc"] = _build()
    nc = _CACHE["nc"]
    in_maps = _in_maps(x, in_proj_w, in_proj_b, out_w, out_b, w1, b1, w2, b2,
                       ln1_g, ln1_b, ln2_g, ln2_b)
    res = run_bass_kernel_spmd(nc, in_maps, core_ids=list(range(8)))
    out = np.empty((B, S, D), dtype=np.float32)
    for c in range(8):
        b_, qb = c // 4, c % 4
        out[b_, qb * Q:(qb + 1) * Q, :] = res.results[c]["yT"].T
    return out
